# revision 2
# baseline (speedup 1.0000x reference)
import sys
import numpy as np
import ml_dtypes

sys.path.insert(0, '/opt/trn_rl_repo')

import concourse.bacc as bacc
import concourse.mybir as mybir
from concourse.bass_utils import run_bass_kernel_spmd
from concourse.tile import TileContext
from contextlib import ExitStack

f32 = mybir.dt.float32
bf16 = mybir.dt.bfloat16
AF = mybir.ActivationFunctionType
ALU = mybir.AluOpType

D_MODEL = 1024
N_HEAD = 16
D_HEAD = 64
B = 4
T = 2048
N_CORES = 8
HPC = N_HEAD // 2        # 8 heads per core
HD = HPC * D_HEAD        # 512 head-dims per core
NTK = D_MODEL // 128     # 8 k-chunks over model dim
NTT = T // 128           # 16 T-tiles of 128

_cache = {}


def _build():
    nc = bacc.Bacc()
    xT = nc.declare_dram_parameter("xT", [D_MODEL, T], bf16, isOutput=False)
    wqkT = nc.declare_dram_parameter("wqkT", [D_MODEL, 2 * HD], bf16, isOutput=False)
    wvT = nc.declare_dram_parameter("wvT", [D_MODEL, HD], bf16, isOutput=False)
    wpT = nc.declare_dram_parameter("wpT", [HD, D_MODEL], bf16, isOutput=False)
    trimask = nc.declare_dram_parameter("trimask", [128, 128], bf16, isOutput=False)
    vones = nc.declare_dram_parameter("vones", [128, HPC], bf16, isOutput=False)
    outp = nc.declare_dram_parameter("out", [T, D_MODEL], f32, isOutput=True)

    with TileContext(nc) as tc, ExitStack() as outer:
        # ---- pools ----
        qkp = outer.enter_context(tc.tile_pool(name="qk", bufs=1))
        vp = outer.enter_context(tc.tile_pool(name="v", bufs=1))
        smp = outer.enter_context(tc.tile_pool(name="small", bufs=1))
        ywp = outer.enter_context(tc.tile_pool(name="yw", bufs=1))
        psA = outer.enter_context(tc.tile_pool(name="psA", bufs=2, space="PSUM"))
        psY = outer.enter_context(tc.tile_pool(name="psY", bufs=1, space="PSUM"))
        ptp = outer.enter_context(tc.tile_pool(name="pt", bufs=3))
        rp = outer.enter_context(tc.tile_pool(name="r", bufs=2))
        xp = outer.enter_context(tc.tile_pool(name="x", bufs=1))
        wqp = outer.enter_context(tc.tile_pool(name="wq", bufs=1))
        wvp = outer.enter_context(tc.tile_pool(name="wv", bufs=1))
        ps1 = outer.enter_context(tc.tile_pool(name="ps1", bufs=2, space="PSUM"))
        obp = outer.enter_context(tc.tile_pool(name="ob", bufs=6))

        qk = [qkp.tile([128, T], bf16, tag=f"qk{m}", name=f"qk{m}") for m in range(8)]
        vt = [vp.tile([128, HPC * 65], bf16, tag=f"v{t}", name=f"va{t}")
              for t in range(NTT)]
        mask = smp.tile([128, 128], bf16, name="maskt")
        ynall = ywp.tile([128, NTT, HD], bf16, name="ynall")     # y norm [q, (j, 8h*64)]
        ysbT = ywp.tile([128, HD // 128, T], bf16, name="ysbT")  # y^T [d, slice, t]
        wp = [ywp.tile([128, D_MODEL], bf16, tag=f"wp{k}", name=f"wpt{k}")
              for k in range(HD // 128)]
        xt = [xp.tile([128, T], bf16, tag=f"x{k}", name=f"xt{k}") for k in range(NTK)]
        wq = [wqp.tile([128, 2 * HD], bf16, tag=f"wq{k}", name=f"wqt{k}")
              for k in range(NTK)]
        wv = [wvp.tile([128, HD], bf16, tag=f"wv{k}", name=f"wvt{k}")
              for k in range(NTK)]

        # ---- input DMAs (hw-DGE queues only; deadline-ordered) ----
        # ACT: mask, x quarter 1, wv — then the queue stays clear for exp.
        # SP: wq (q-cols, then k-cols m0 first), x quarters 2-4, wp.
        nc.scalar.dma_start(out=mask[:], in_=trimask[:, :])
        for k in range(NTK):
            nc.scalar.dma_start(out=xt[k][:, 0:512],
                                in_=xT[k * 128:(k + 1) * 128, 0:512])
        for k in range(NTK):
            nc.scalar.dma_start(out=wv[k][:], in_=wvT[k * 128:(k + 1) * 128, :])
        for k in range(NTK):
            nc.sync.dma_start(out=wq[k][:, 0:512],
                              in_=wqkT[k * 128:(k + 1) * 128, 0:512])
        for k in range(NTK):
            nc.sync.dma_start(out=wq[k][:, 512:640],
                              in_=wqkT[k * 128:(k + 1) * 128, 512:640])
        for k in range(NTK):
            nc.sync.dma_start(out=xt[k][:, 512:1024],
                              in_=xT[k * 128:(k + 1) * 128, 512:1024])
        for k in range(NTK):
            nc.sync.dma_start(out=wq[k][:, 640:1024],
                              in_=wqkT[k * 128:(k + 1) * 128, 640:1024])
        for qtr in (2, 3):
            for k in range(NTK):
                nc.sync.dma_start(out=xt[k][:, 512 * qtr:512 * (qtr + 1)],
                                  in_=xT[k * 128:(k + 1) * 128,
                                         512 * qtr:512 * (qtr + 1)])
        for k in range(HD // 128):
            nc.sync.dma_start(out=wp[k][:], in_=wpT[k * 128:(k + 1) * 128, :])
        # warm the ACT exp table early (one-time table-load off critical path)
        warm = smp.tile([1, 16], f32, name="warmt")
        nc.vector.memset(warm[:], 0.0)
        nc.scalar.activation(warm[:], warm[:], AF.Exp)
        # warm the GPSIMD ucode library (~70us lazy load on first compute op)
        gwarm = smp.tile([2, 16], bf16, name="gwarmt")
        nc.gpsimd.memset(gwarm[:], 1.0)
        nc.gpsimd.tensor_tensor(gwarm[:], gwarm[:], gwarm[:], ALU.mult)

        # ---- S1 work units ----
        s1a_done = set()
        s1b_done = set()

        def emit_s1a(mq, j):
            if (mq, j) in s1a_done:
                return
            s1a_done.add((mq, j))
            ps = ps1.tile([128, 512], f32, tag="ps1", name="ps1t")
            for k in range(NTK):
                nc.tensor.matmul(ps[:], wq[k][:, mq * 128:(mq + 1) * 128],
                                 xt[k][:, j * 512:(j + 1) * 512],
                                 start=(k == 0), stop=(k == NTK - 1))
            nc.vector.tensor_copy(qk[mq][:, j * 512:(j + 1) * 512], ps[:])

        def emit_s1b(t):
            if t in s1b_done:
                return
            s1b_done.add(t)
            va3 = vt[t][:].rearrange("p (h e) -> p h e", e=65)
            nc.vector.memset(va3[:, :, 64], 1.0)
            ps = ps1.tile([128, HD], f32, tag="ps1", name="ps1vt")
            for k in range(NTK):
                nc.tensor.matmul(ps[:], xt[k][:, t * 128:(t + 1) * 128], wv[k][:, :],
                                 start=(k == 0), stop=(k == NTK - 1))
            nc.vector.tensor_copy(
                va3[:, :, 0:64], ps[:].rearrange("p (h e) -> p h e", e=64))

        # S4 units, emitted one jc-round late so transposes have landed
        def emit_s4(j):
            for oc in range(2):
                ps = ps1.tile([128, 512], f32, tag="ps1", name="ps4t")
                for k in range(HD // 128):
                    nc.tensor.matmul(
                        ps[:], ysbT[:, k, 128 * j:128 * j + 128],
                        wp[k][:, oc * 512:(oc + 1) * 512],
                        start=(k == 0), stop=(k == HD // 128 - 1))
                o_ = obp.tile([128, 512], f32, tag="o", name="obt")
                nc.vector.tensor_copy(o_[:], ps[:])
                nc.sync.dma_start(
                    out=outp[j * 128:(j + 1) * 128, oc * 512:(oc + 1) * 512],
                    in_=o_[:])

        def norm_one(jc, jj, hA, hB, psy3A, psy3B):
            # y[q, d] = psy[q, d] / psy[q, 64] for one q-tile, both heads
            j = 4 * jc + jj
            rA = rp.tile([128, 1], f32, tag="rA", name="rAt")
            rB = rp.tile([128, 1], f32, tag="rB", name="rBt")
            nc.vector.reciprocal(rA[:], psy3A[:, jj, 64:65])
            nc.vector.reciprocal(rB[:], psy3B[:, jj, 64:65])
            nc.vector.tensor_scalar_mul(
                ynall[:, j, 64 * hA:64 * hA + 64], psy3A[:, jj, 0:64], rA[:])
            nc.vector.tensor_scalar_mul(
                ynall[:, j, 64 * hB:64 * hB + 64], psy3B[:, jj, 0:64], rB[:])

        # ---- main: jc outer (512-query chunks), head-pair m inner ----
        s4_pending = []
        for jc in range(4):
            for m in range(4):
                hA, hB = 2 * m, 2 * m + 1
                emit_s1a(m, jc)      # qt columns for this round
                qt, kt = qk[m], qk[4 + m]
                qlo = 512 * jc
                # PE filler units interleaved into the i-loop (each with an
                # emission deadline), so ACT's exp stream starts immediately
                # at round start and jc=3's exp-bound rounds keep PE fed
                fillers = []
                if m == 0:
                    for t in range(4 * jc, 4 * jc + 4):
                        fillers.append((t, ("s1b", t)))
                fillers.append((4 * jc, ("s1a", 4 + m, jc)))
                ns4 = [2, 2, 3, 3][m] if jc == 3 else \
                    (1 if (jc == 2 and m >= 2) else 0)
                for _ in range(ns4):
                    if s4_pending:
                        fillers.append((10 ** 9, ("s4", s4_pending.pop(0))))
                fillers.sort(key=lambda f: f[0])

                def emit_unit(u):
                    if u[0] == "s1b":
                        emit_s1b(u[1])
                    elif u[0] == "s1a":
                        emit_s1a(u[1], u[2])
                    else:
                        emit_s4(u[1])

                def drain_fillers(i, force_all=False):
                    opportunistic = (i % 4 == 3) if jc == 3 else (i % 2 == 1)
                    while fillers:
                        dl, u = fillers[0]
                        if force_all or dl <= i + 1 or opportunistic:
                            fillers.pop(0)
                            emit_unit(u)
                            opportunistic = False
                        else:
                            break
                # one PSUM bank per head; 4 q-tile slices of 65 cols share the
                # bank as one accumulation group
                psyA = psY.tile([128, 512], f32, tag="psyA", name="psyAt")
                psyB = psY.tile([128, 512], f32, tag="psyB", name="psyBt")
                psy3A = psyA[:, 0:260].rearrange("p (j c) -> p j c", c=65)
                psy3B = psyB[:, 0:260].rearrange("p (j c) -> p j c", c=65)
                # accumulate onto DVE-zeroed psum without HW start/stop groups,
                # so completed q-tile slices can be read out mid-round
                nc.vector.memset(psyA[:, 0:260], 0.0)
                nc.vector.memset(psyB[:, 0:260], 0.0)
                imax = 4 * jc + 4
                for i in range(imax):
                    drain_fillers(i)
                    su = max(0, 128 * i - qlo)
                    psa = psA.tile([128, 1024], f32, tag="psa", name="psat")
                    nc.tensor.matmul(
                        psa[:, su:512], kt[0:64, i * 128:(i + 1) * 128],
                        qt[0:64, qlo + su:qlo + 512], start=True, stop=True)
                    nc.tensor.matmul(
                        psa[:, 512 + su:1024], kt[64:128, i * 128:(i + 1) * 128],
                        qt[64:128, qlo + su:qlo + 512], start=True, stop=True)
                    pt = ptp.tile([128, 1024], bf16, tag="pt", name="ptile")
                    p3i = psa[:].rearrange("p (g c) -> p g c", g=2)
                    p3o = pt[:].rearrange("p (g c) -> p g c", g=2)
                    nc.scalar.activation(p3o[:, :, su:512], p3i[:, :, su:512], AF.Exp)
                    if 128 * i >= qlo:  # diagonal block: mask keys > query
                        nc.gpsimd.tensor_tensor(
                            pt[:, su:su + 128], pt[:, su:su + 128], mask[:], ALU.mult)
                        nc.gpsimd.tensor_tensor(
                            pt[:, 512 + su:512 + su + 128],
                            pt[:, 512 + su:512 + su + 128], mask[:], ALU.mult)
                    jj0 = max(0, i - 4 * jc)
                    for jj in range(jj0, 4):
                        nc.tensor.matmul(
                            psy3A[:, jj, :], pt[:, 128 * jj:128 * jj + 128],
                            vt[i][:, 65 * hA:65 * hA + 65],
                            start=False, stop=False, skip_group_check=True)
                        nc.tensor.matmul(
                            psy3B[:, jj, :], pt[:, 512 + 128 * jj:512 + 128 * jj + 128],
                            vt[i][:, 65 * hB:65 * hB + 65],
                            start=False, stop=False, skip_group_check=True)
                    if jc == 3 and m == 3 and i >= 12:
                        # psy slice jj is complete at i == j: stream out
                        # normalize + transpose + projection per q-tile so the
                        # kernel tail collapses
                        jj = i - 12
                        j = 12 + jj
                        norm_one(jc, jj, hA, hB, psy3A, psy3B)
                        nc.sync.dma_start_transpose(
                            out=ysbT[:, :, 128 * j:128 * (j + 1)],
                            in_=ynall[:, j, :])
                        emit_s4(j)

                drain_fillers(imax, force_all=True)
                if not (jc == 3 and m == 3):
                    # normalize: y[q, d] = psy[q, d] / psy[q, 64]
                    for jj in range(4):
                        norm_one(jc, jj, hA, hB, psy3A, psy3B)

            # jc round complete for all heads: transpose y for these q-tiles
            if jc < 3:
                for jj in range(4):
                    j = 4 * jc + jj
                    nc.sync.dma_start_transpose(
                        out=ysbT[:, :, 128 * j:128 * (j + 1)], in_=ynall[:, j, :])
                    s4_pending.append(j)

        for j in s4_pending:
            emit_s4(j)

    nc.compile()
    return nc


def _prep_core_inputs(x, w_qkv, w_proj, c):
    b, g = c // 2, c % 2
    scale = np.float32(D_HEAD ** -0.5)
    wq = (w_qkv[g * HD:(g + 1) * HD] * scale)
    wk = w_qkv[D_MODEL + g * HD:D_MODEL + (g + 1) * HD]
    wv = w_qkv[2 * D_MODEL + g * HD:2 * D_MODEL + (g + 1) * HD]
    tri = np.triu(np.ones((128, 128), dtype=np.float32))
    bf = ml_dtypes.bfloat16
    return {
        "xT": np.ascontiguousarray(x[b].T).astype(bf),
        "wqkT": np.ascontiguousarray(np.concatenate([wq, wk], 0).T).astype(bf),
        "wvT": np.ascontiguousarray(wv.T).astype(bf),
        "wpT": np.ascontiguousarray(w_proj[:, g * HD:(g + 1) * HD].T).astype(bf),
        "trimask": tri.astype(bf),
        "vones": np.ones((128, HPC), dtype=np.float32).astype(bf),
    }


def kernel(x, w_qkv, w_proj):
    x = np.asarray(x)
    w_qkv = np.asarray(w_qkv)
    w_proj = np.asarray(w_proj)
    if "nc" not in _cache:
        _cache["nc"] = _build()
    nc = _cache["nc"]
    in_maps = [_prep_core_inputs(x, w_qkv, w_proj, c) for c in range(N_CORES)]
    res = run_bass_kernel_spmd(nc, in_maps, core_ids=list(range(N_CORES)))
    outs = [res.results[c]["out"] for c in range(N_CORES)]
    return np.stack([outs[2 * b] + outs[2 * b + 1] for b in range(B)], 0)
